# revision 34
# baseline (speedup 1.0000x reference)
"""v8: quad-unit bucketed on-device gather (ap_gather, 8 groups, 4 edges/unit).

Host (untimed prep): per core, sort edges by src and pack runs of equal
src into quad gather-units (up to 4 edges per unit; avg src multiplicity
is 2M/500k = 4), bucketed by d-slice q = src // W with W = 3907 =
ceil(N/128) so all 128 buckets are equally loaded (~5350 +- 50 units;
capacity 5696, host fallback beyond). Partition 16g+c holds
d[W*(16g+c) : +4096]; the full table is 2MB in SBUF with no replication.
Group g processes its 16 buckets over 16 channel steps; in call t
(channel c = t//B, static schedule) every group's gather output row
16g + c is exactly d[src] for its units -- no candidate select. The
compact DMA expands each gathered row 4x via partition_broadcast so the
DVE multiply sees one (val, d[src]) pair per edge.

Device, per call: DMA gidx/vals in -> ap_gather (128 channels) -> compact
strided rows {16g+c} to partitions 0..31 (x4 expand) via SBUF-SBUF DMA -> DVE multiply
by vals (bf16) -> DMA contrib out. 3-deep buffer pipeline keeps the gather
engine (~41 ns/idx, the bottleneck) busy.

Host: final np.add.at segment-sum + masked L1 (no device scatter primitive).
"""
import sys
sys.path.insert(0, "/opt/trn_rl_repo")
import numpy as np

N_NODES = 500_000
N_EDGES = 16_000_000
N_CORES = 8
E_CORE = N_EDGES // N_CORES          # 2_000_000
G = 8                                 # gpsimd groups (16 partitions each)
NI = 2848                             # gather units per group per call
NBUCK = 128                           # d-slices, one per partition
SLICE = 4096                          # table elems per partition (>= W + max lo)
W = 3907                              # slice width: ceil(N_NODES/128) balances buckets
B = 2                                 # calls per channel step
DUP = 4                               # edges packed per gather unit (same src)
ROWS = DUP * 8                        # 32 expanded partitions
CAPU = B * NI                         # 5696 unit slots per bucket (max seen 5472)
NCALLS = 16 * B                       # 64
S16 = NI // 16                        # idx columns per call
BUFS = 4
_RUNNER2 = None


def _build():
    import concourse.bass as bass
    import concourse.bacc as bacc
    import concourse.mybir as mybir
    from concourse import library_config

    nc = bacc.Bacc(None, target_bir_lowering=False)
    dtab = nc.dram_tensor("dtab", [128, SLICE], mybir.dt.float32, kind="ExternalInput")
    gidx = nc.dram_tensor("gidx", [128, NCALLS * S16], mybir.dt.int16, kind="ExternalInput")
    vals = nc.dram_tensor("vals", [ROWS, NCALLS * NI], mybir.dt.bfloat16, kind="ExternalInput")
    contrib = nc.dram_tensor("contrib", [ROWS, NCALLS * NI], mybir.dt.bfloat16, kind="ExternalOutput")

    with (
        nc.Block() as block,
        nc.semaphore("s_const") as s_const,
        nc.semaphore("s_gi") as s_gi,
        nc.semaphore("s_va") as s_va,
        nc.semaphore("s_gth") as s_gth,
        nc.semaphore("s_cp") as s_cp,
        nc.semaphore("s_mu") as s_mu,
        nc.semaphore("s_out") as s_out,
        nc.sbuf_tensor("dtab_sb", [128, SLICE], mybir.dt.float32) as dtab_sb,
        nc.sbuf_tensor("gi_sb", [128, BUFS * S16], mybir.dt.int16) as gi_sb,
        nc.sbuf_tensor("va_sb", [ROWS, BUFS * NI], mybir.dt.bfloat16) as va_sb,
        nc.sbuf_tensor("ga_sb", [128, BUFS * NI], mybir.dt.float32) as ga_sb,
        nc.sbuf_tensor("cp_sb", [ROWS, BUFS * NI], mybir.dt.float32) as cp_sb,
        nc.sbuf_tensor("ct_sb", [ROWS, BUFS * NI], mybir.dt.bfloat16) as ct_sb,
    ):
        def kof(t):
            return t // B

        @block.scalar
        def _(scalar):
            for t in range(NCALLS):
                b = t % BUFS
                if t >= BUFS:
                    # gi_sb[b] last read by gather t-BUFS; va_sb[b] by mult t-BUFS
                    scalar.wait_ge(s_gth, t - BUFS + 1)
                    scalar.wait_ge(s_mu, t - BUFS + 1)
                scalar.dma_start(
                    gi_sb[:, b * S16:(b + 1) * S16],
                    gidx.ap()[:, t * S16:(t + 1) * S16],
                ).then_inc(s_gi, 16)
                scalar.dma_start(
                    va_sb[:, b * NI:(b + 1) * NI],
                    vals.ap()[:, t * NI:(t + 1) * NI],
                ).then_inc(s_va, 16)

        @block.sync
        def _(sync):
            sync.dma_start(dtab_sb[:, :], dtab.ap()).then_inc(s_const, 16)
            for u in range(NCALLS):
                bu = u % BUFS
                sync.wait_ge(s_gth, u + 1)            # gather u done
                if u >= BUFS:
                    sync.wait_ge(s_mu, u - BUFS + 1)  # cp_sb[bu] free
                for dd in range(DUP):
                    sync.dma_start(
                        cp_sb[8 * dd:8 * dd + 8, bu * NI:(bu + 1) * NI],
                        ga_sb[kof(u)::16, bu * NI:(bu + 1) * NI],
                    ).then_inc(s_cp, 16)
                if u >= 1:
                    v = u - 1
                    bv = v % BUFS
                    sync.wait_ge(s_mu, v + 1)         # mult v done
                    sync.dma_start(
                        contrib.ap()[:, v * NI:(v + 1) * NI],
                        ct_sb[:, bv * NI:(bv + 1) * NI],
                    ).then_inc(s_out, 16)
            v = NCALLS - 1
            sync.wait_ge(s_mu, v + 1)
            sync.dma_start(
                contrib.ap()[:, v * NI:(v + 1) * NI],
                ct_sb[:, (v % BUFS) * NI:((v % BUFS) + 1) * NI],
            ).then_inc(s_out, 16)
            sync.wait_ge(s_out, 16 * NCALLS)

        @block.gpsimd
        def _(g):
            g.load_library(library_config.ap_gather)
            for t in range(NCALLS):
                b = t % BUFS
                g.wait_ge(s_const, 16)               # table resident
                g.wait_ge(s_gi, 16 * (t + 1))        # gidx t landed
                if t >= BUFS:
                    g.wait_ge(s_cp, 16 * DUP * (t - BUFS + 1))  # ga_sb[b] compacted
                g.ap_gather(
                    out_ap=ga_sb[:, b * NI:(b + 1) * NI].rearrange(
                        "p (n d) -> p n d", d=1),
                    in_ap=dtab_sb[:, :].rearrange("p (n d) -> p n d", d=1),
                    idxs_ap=gi_sb[:, b * S16:(b + 1) * S16],
                    channels=128, num_elems=SLICE, d=1, num_idxs=NI,
                ).then_inc(s_gth, 1)

        @block.vector
        def _(vector):
            for t in range(NCALLS):
                b = t % BUFS
                vector.wait_ge(s_cp, 16 * DUP * (t + 1))   # compact t done
                vector.wait_ge(s_va, 16 * (t + 1))   # vals t landed
                if t >= BUFS:
                    vector.wait_ge(s_out, 16 * (t - BUFS + 1))  # ct_sb[b] free
                vector.tensor_tensor(
                    out=ct_sb[:, b * NI:(b + 1) * NI],
                    in0=cp_sb[:, b * NI:(b + 1) * NI],
                    in1=va_sb[:, b * NI:(b + 1) * NI],
                    op=mybir.AluOpType.mult,
                ).then_inc(s_mu, 1)

    nc.finalize()
    return nc


# ---- embedded SPMD runner ----
import time
import numpy as np
import jax
from jax.sharding import Mesh, PartitionSpec
from jax.experimental.shard_map import shard_map

import concourse.bass as bass
import concourse.mybir as mybir
from concourse import bass2jax
from concourse.bass2jax import _bass_exec_p, install_neuronx_cc_hook, partition_id_tensor


class SpmdRunner:
    def __init__(self, nc, n_cores=8):
        install_neuronx_cc_hook()
        self.nc = nc
        self.n_cores = n_cores
        assert nc.dbg_addr is None or not nc.dbg_callbacks
        partition_name = nc.partition_id_tensor.name if nc.partition_id_tensor else None
        in_names, out_names, out_avals, zero_outs = [], [], [], []
        for alloc in nc.m.functions[0].allocations:
            if not isinstance(alloc, mybir.MemoryLocationSet):
                continue
            name = alloc.memorylocations[0].name
            if alloc.kind == "ExternalInput":
                if name != partition_name and name != (nc.dbg_addr.name if nc.dbg_addr else None):
                    in_names.append(name)
            elif alloc.kind == "ExternalOutput":
                out_names.append(name)
                shape = tuple(alloc.tensor_shape)
                dtype = mybir.dt.np(alloc.dtype)
                out_avals.append(jax.core.ShapedArray(shape, dtype))
                zero_outs.append(np.zeros(shape, dtype))
        self.in_names, self.out_names = in_names, out_names
        self.out_avals, self.zero_outs = out_avals, zero_outs
        n_params, n_outs = len(in_names), len(out_avals)
        self.n_params = n_params

        all_in_names = list(in_names) + list(out_names)
        if nc.dbg_addr is not None:
            self.dbg_name = nc.dbg_addr.name
        else:
            self.dbg_name = None
        if partition_name is not None:
            all_in_names.append(partition_name)

        def _body(*args):
            operands = list(args)
            if partition_name is not None:
                operands.append(partition_id_tensor())
            outs = _bass_exec_p.bind(
                *operands,
                out_avals=tuple(out_avals),
                in_names=tuple(all_in_names),
                out_names=tuple(out_names),
                lowering_input_output_aliases=(),
                sim_require_finite=True,
                sim_require_nnan=True,
                nc=nc,
            )
            return tuple(outs)

        devices = jax.devices()[:n_cores]
        self.mesh = Mesh(np.asarray(devices), ("core",))
        in_specs = (PartitionSpec("core"),) * (n_params + n_outs)
        out_specs = (PartitionSpec("core"),) * n_outs
        # no donation so we can re-run with cached device inputs
        self.fn = jax.jit(
            shard_map(_body, mesh=self.mesh, in_specs=in_specs,
                      out_specs=out_specs, check_rep=False),
            keep_unused=True,
        )
        self._cached_dev_in = None

    def put_inputs(self, in_maps):
        """in_maps: list of n_cores dicts name->np array. Returns device arrays."""
        concat = [
            np.concatenate([np.asarray(in_maps[c][n]) for c in range(self.n_cores)], axis=0)
            for n in self.in_names
        ]
        concat += [
            np.zeros((self.n_cores * z.shape[0], *z.shape[1:]), z.dtype)
            for z in self.zero_outs
        ]
        self._cached_dev_in = jax.device_put(concat)
        return self._cached_dev_in

    def run(self, dev_in=None):
        dev_in = dev_in if dev_in is not None else self._cached_dev_in
        outs = self.fn(*dev_in)
        jax.block_until_ready(outs)
        return outs

    def results(self, outs):
        res = []
        for c in range(self.n_cores):
            m = {}
            for i, name in enumerate(self.out_names):
                a = np.asarray(outs[i]).reshape(self.n_cores, *self.out_avals[i].shape)
                m[name] = a[c]
            res.append(m)
        return res

    def time_runs(self, reps=5):
        ts = []
        for _ in range(reps):
            t0 = time.perf_counter()
            self.run()
            ts.append(time.perf_counter() - t0)
        return min(ts), ts


def _get_runner():
    global _RUNNER2
    if _RUNNER2 is None:
        _RUNNER2 = SpmdRunner(_build(), N_CORES)
    return _RUNNER2

_get_runner2 = _get_runner


def _prep_core(src, dstv, valv):
    """Pack the core's edges into quad gather-units (up to DUP edges sharing
    one src per unit), bucketed by d-slice. Returns device in_map pieces,
    the dst layout matching the device contrib layout, and overflow edges."""
    import concourse.mybir as mybir
    bf16 = mybir.dt.np(mybir.dt.bfloat16)

    order = np.argsort(src, kind="stable")   # sort by src == (bucket, lo)
    s = src[order]
    v = valv[order]
    dd = dstv[order]
    ne = len(s)
    first = np.concatenate([[True], s[1:] != s[:-1]])
    run_start = np.flatnonzero(first)
    run_id = np.cumsum(first) - 1
    rank = np.arange(ne) - run_start[run_id]
    dup = (rank & (DUP - 1)).astype(np.int64)
    is_u = dup == 0
    k = s // W
    unit_gid = np.cumsum(is_u) - 1
    ucnt = np.bincount(k[is_u], minlength=NBUCK)
    uoff = np.concatenate([[0], np.cumsum(ucnt)[:-1]])
    u_in_b = unit_gid - uoff[k]
    ok = u_in_b < CAPU
    lo = (s - k * W).astype(np.int16)

    idx_arr = np.zeros((NBUCK, CAPU), np.int16)
    val_arr = np.zeros((NBUCK, DUP, CAPU), np.float32)
    dst_arr = np.zeros((NBUCK, DUP, CAPU), np.int32)
    mu = ok & is_u
    idx_arr[k[mu], u_in_b[mu]] = lo[mu]
    val_arr[k[ok], dup[ok], u_in_b[ok]] = v[ok]
    dst_arr[k[ok], dup[ok], u_in_b[ok]] = dd[ok]

    # bucket 16g + c is processed by group g at channel step c; call
    # t = c*B + bb takes unit slots [bb*NI, (bb+1)*NI) of each bucket
    T1 = np.ascontiguousarray(
        idx_arr.reshape(G, 16, B, NI).transpose(1, 2, 0, 3)
    ).reshape(NCALLS, G, NI)
    X = T1.reshape(NCALLS, G, S16, 16)
    gidx = np.ascontiguousarray(X.transpose(1, 3, 0, 2)).reshape(128, NCALLS * S16)

    def rows_layout(a):  # [NBUCK, DUP, CAPU] -> [ROWS, NCALLS*NI], row=dup*8+g
        return np.ascontiguousarray(
            a.reshape(G, 16, DUP, B, NI).transpose(2, 0, 1, 3, 4)
        ).reshape(ROWS, NCALLS * NI)

    vals = rows_layout(val_arr).astype(bf16)
    dstb = rows_layout(dst_arr)
    ov = order[~ok]
    return {"gidx": gidx, "vals": vals}, dstb, ov


def kernel(d, edge_index, matrix_values, mask, residual):
    d = np.asarray(d, dtype=np.float32)
    edge_index = np.asarray(edge_index)
    matrix_values = np.asarray(matrix_values, dtype=np.float32)
    mask = np.asarray(mask)
    residual = np.asarray(residual, dtype=np.float32)
    dst = edge_index[0].astype(np.int32)
    src = edge_index[1].astype(np.int32)
    d_ext = np.concatenate(
        [d, np.zeros(127 * W + SLICE - N_NODES, np.float32)])
    dtab_host = d_ext[
        W * np.arange(128)[:, None] + np.arange(SLICE)[None, :]]

    in_maps, dst_blocks, overflow = [], [], []
    for c in range(N_CORES):
        sl = slice(c * E_CORE, (c + 1) * E_CORE)
        m, dstb, ov = _prep_core(src[sl], dst[sl], matrix_values[sl])
        m["dtab"] = dtab_host
        in_maps.append(m)
        dst_blocks.append(dstb)
        if len(ov):
            overflow.append((c, ov))

    r = _get_runner2()
    r.put_inputs(in_maps)
    outs = r.run()
    res = r.results(outs)

    Ad = np.zeros(N_NODES, np.float32)
    for c in range(N_CORES):
        ctb = res[c]["contrib"].astype(np.float32)   # [ROWS, NCALLS*NI]
        np.add.at(Ad, dst_blocks[c].ravel(), ctb.ravel())
    for c, ov in overflow:  # safety net: never taken for the target input
        sl = slice(c * E_CORE, (c + 1) * E_CORE)
        s_, d_, v_ = src[sl][ov], dst[sl][ov], matrix_values[sl][ov]
        np.add.at(Ad, d_, v_ * d[s_])
    Ad = np.where(mask, Ad, np.float32(0))
    return np.asarray(np.mean(np.abs(Ad - residual)), dtype=np.float32)


# revision 36
# speedup vs baseline: 1.0273x; 1.0273x over previous
"""v8: quad-unit bucketed on-device gather (ap_gather, 8 groups, 4 edges/unit).

Host (untimed prep): per core, sort edges by src and pack runs of equal
src into quad gather-units (up to 4 edges per unit; avg src multiplicity
is 2M/500k = 4), bucketed by d-slice q = src // W with W = 3907 =
ceil(N/128) so all 128 buckets are equally loaded (~5350 +- 50 units;
capacity 5696, host fallback beyond). Partition 16g+c holds
d[W*(16g+c) : +4096]; the full table is 2MB in SBUF with no replication.
Group g processes its 16 buckets over 16 channel steps; in call t
(channel c = t//B, static schedule) every group's gather output row
16g + c is exactly d[src] for its units -- no candidate select. The
compact DMAs replicate each gathered row into 4 dup blocks so the
DVE multiply sees one (val, d[src]) pair per edge.

Device, per call: DMA gidx/vals in -> ap_gather (128 channels) -> compact
strided rows {16g+c} to partitions 0..31 (x4 expand) via SBUF-SBUF DMAs -> DVE
multiply by vals (bf16) -> DMA contrib out. 4-deep buffer pipeline keeps the gather
engine (~41 ns/idx, the bottleneck) busy.

Host: final np.add.at segment-sum + masked L1 (no device scatter primitive).
"""
import sys
sys.path.insert(0, "/opt/trn_rl_repo")
import numpy as np

N_NODES = 500_000
N_EDGES = 16_000_000
N_CORES = 8
E_CORE = N_EDGES // N_CORES          # 2_000_000
G = 8                                 # gpsimd groups (16 partitions each)
NI = 1424                             # gather units per group per call
NBUCK = 128                           # d-slices, one per partition
SLICE = 4096                          # table elems per partition (>= W + max lo)
W = 3907                              # slice width: ceil(N_NODES/128) balances buckets
B = 4                                 # calls per channel step
DUP = 4                               # edges packed per gather unit (same src)
ROWS = DUP * 8                        # 32 expanded partitions
CAPU = B * NI                         # 5696 unit slots per bucket (max seen 5472)
NCALLS = 16 * B                       # 64
S16 = NI // 16                        # idx columns per call
BUFS = 4
_RUNNER2 = None


def _build():
    import concourse.bass as bass
    import concourse.bacc as bacc
    import concourse.mybir as mybir
    from concourse import library_config

    nc = bacc.Bacc(None, target_bir_lowering=False)
    dtab = nc.dram_tensor("dtab", [128, SLICE], mybir.dt.float32, kind="ExternalInput")
    gidx = nc.dram_tensor("gidx", [128, NCALLS * S16], mybir.dt.int16, kind="ExternalInput")
    vals = nc.dram_tensor("vals", [G, NCALLS * DUP * NI], mybir.dt.bfloat16, kind="ExternalInput")
    contrib = nc.dram_tensor("contrib", [G, NCALLS * DUP * NI], mybir.dt.bfloat16, kind="ExternalOutput")

    with (
        nc.Block() as block,
        nc.semaphore("s_const") as s_const,
        nc.semaphore("s_gi") as s_gi,
        nc.semaphore("s_va") as s_va,
        nc.semaphore("s_gth") as s_gth,
        nc.semaphore("s_cp") as s_cp,
        nc.semaphore("s_mu") as s_mu,
        nc.semaphore("s_out") as s_out,
        nc.sbuf_tensor("dtab_sb", [128, SLICE], mybir.dt.float32) as dtab_sb,
        nc.sbuf_tensor("gi_sb", [128, BUFS * S16], mybir.dt.int16) as gi_sb,
        nc.sbuf_tensor("va_sb", [G, BUFS * DUP * NI], mybir.dt.bfloat16) as va_sb,
        nc.sbuf_tensor("ga_sb", [128, BUFS * NI], mybir.dt.float32) as ga_sb,
        nc.sbuf_tensor("cp_sb", [G, BUFS * NI], mybir.dt.float32) as cp_sb,
        nc.sbuf_tensor("ct_sb", [G, BUFS * DUP * NI], mybir.dt.bfloat16) as ct_sb,
    ):
        def kof(t):
            return t // B

        @block.scalar
        def _(scalar):
            for t in range(NCALLS):
                b = t % BUFS
                if t >= BUFS:
                    # gi_sb[b] last read by gather t-BUFS; va_sb[b] by mult t-BUFS
                    scalar.wait_ge(s_gth, t - BUFS + 1)
                    scalar.wait_ge(s_mu, t - BUFS + 1)
                scalar.dma_start(
                    gi_sb[:, b * S16:(b + 1) * S16],
                    gidx.ap()[:, t * S16:(t + 1) * S16],
                ).then_inc(s_gi, 16)
                scalar.dma_start(
                    va_sb[:, b * DUP * NI:(b + 1) * DUP * NI],
                    vals.ap()[:, t * DUP * NI:(t + 1) * DUP * NI],
                ).then_inc(s_va, 16)

        @block.sync
        def _(sync):
            sync.dma_start(dtab_sb[:, :], dtab.ap()).then_inc(s_const, 16)
            for u in range(NCALLS):
                bu = u % BUFS
                sync.wait_ge(s_gth, u + 1)            # gather u done
                if u >= BUFS:
                    sync.wait_ge(s_mu, u - BUFS + 1)  # cp_sb[bu] free
                sync.dma_start(
                    cp_sb[:, bu * NI:(bu + 1) * NI],
                    ga_sb[kof(u)::16, bu * NI:(bu + 1) * NI],
                ).then_inc(s_cp, 16)
                if u >= 1:
                    v = u - 1
                    bv = v % BUFS
                    sync.wait_ge(s_mu, v + 1)         # mult v done
                    sync.dma_start(
                        contrib.ap()[:, v * DUP * NI:(v + 1) * DUP * NI],
                        ct_sb[:, bv * DUP * NI:(bv + 1) * DUP * NI],
                    ).then_inc(s_out, 16)
            v = NCALLS - 1
            sync.wait_ge(s_mu, v + 1)
            sync.dma_start(
                contrib.ap()[:, v * DUP * NI:(v + 1) * DUP * NI],
                ct_sb[:, (v % BUFS) * DUP * NI:((v % BUFS) + 1) * DUP * NI],
            ).then_inc(s_out, 16)
            sync.wait_ge(s_out, 16 * NCALLS)

        @block.gpsimd
        def _(g):
            g.load_library(library_config.ap_gather)
            for t in range(NCALLS):
                b = t % BUFS
                g.wait_ge(s_const, 16)               # table resident
                g.wait_ge(s_gi, 16 * (t + 1))        # gidx t landed
                if t >= BUFS:
                    g.wait_ge(s_cp, 16 * (t - BUFS + 1))  # ga_sb[b] compacted
                g.ap_gather(
                    out_ap=ga_sb[:, b * NI:(b + 1) * NI].rearrange(
                        "p (n d) -> p n d", d=1),
                    in_ap=dtab_sb[:, :].rearrange("p (n d) -> p n d", d=1),
                    idxs_ap=gi_sb[:, b * S16:(b + 1) * S16],
                    channels=128, num_elems=SLICE, d=1, num_idxs=NI,
                ).then_inc(s_gth, 1)

        @block.vector
        def _(vector):
            for t in range(NCALLS):
                b = t % BUFS
                vector.wait_ge(s_cp, 16 * (t + 1))   # compact t done
                vector.wait_ge(s_va, 16 * (t + 1))   # vals t landed
                if t >= BUFS:
                    vector.wait_ge(s_out, 16 * (t - BUFS + 1))  # ct_sb[b] free
                vector.tensor_tensor(
                    out=ct_sb[:, b * DUP * NI:(b + 1) * DUP * NI].rearrange(
                        "p (d n) -> p d n", n=NI),
                    in0=cp_sb[:, b * NI:(b + 1) * NI].rearrange(
                        "p (o n) -> p o n", o=1).to_broadcast([G, DUP, NI]),
                    in1=va_sb[:, b * DUP * NI:(b + 1) * DUP * NI].rearrange(
                        "p (d n) -> p d n", n=NI),
                    op=mybir.AluOpType.mult,
                ).then_inc(s_mu, 1)

    nc.finalize()
    return nc


# ---- embedded SPMD runner ----
import time
import numpy as np
import jax
from jax.sharding import Mesh, PartitionSpec
from jax.experimental.shard_map import shard_map

import concourse.bass as bass
import concourse.mybir as mybir
from concourse import bass2jax
from concourse.bass2jax import _bass_exec_p, install_neuronx_cc_hook, partition_id_tensor


class SpmdRunner:
    def __init__(self, nc, n_cores=8):
        install_neuronx_cc_hook()
        self.nc = nc
        self.n_cores = n_cores
        assert nc.dbg_addr is None or not nc.dbg_callbacks
        partition_name = nc.partition_id_tensor.name if nc.partition_id_tensor else None
        in_names, out_names, out_avals, zero_outs = [], [], [], []
        for alloc in nc.m.functions[0].allocations:
            if not isinstance(alloc, mybir.MemoryLocationSet):
                continue
            name = alloc.memorylocations[0].name
            if alloc.kind == "ExternalInput":
                if name != partition_name and name != (nc.dbg_addr.name if nc.dbg_addr else None):
                    in_names.append(name)
            elif alloc.kind == "ExternalOutput":
                out_names.append(name)
                shape = tuple(alloc.tensor_shape)
                dtype = mybir.dt.np(alloc.dtype)
                out_avals.append(jax.core.ShapedArray(shape, dtype))
                zero_outs.append(np.zeros(shape, dtype))
        self.in_names, self.out_names = in_names, out_names
        self.out_avals, self.zero_outs = out_avals, zero_outs
        n_params, n_outs = len(in_names), len(out_avals)
        self.n_params = n_params

        all_in_names = list(in_names) + list(out_names)
        if nc.dbg_addr is not None:
            self.dbg_name = nc.dbg_addr.name
        else:
            self.dbg_name = None
        if partition_name is not None:
            all_in_names.append(partition_name)

        def _body(*args):
            operands = list(args)
            if partition_name is not None:
                operands.append(partition_id_tensor())
            outs = _bass_exec_p.bind(
                *operands,
                out_avals=tuple(out_avals),
                in_names=tuple(all_in_names),
                out_names=tuple(out_names),
                lowering_input_output_aliases=(),
                sim_require_finite=True,
                sim_require_nnan=True,
                nc=nc,
            )
            return tuple(outs)

        devices = jax.devices()[:n_cores]
        self.mesh = Mesh(np.asarray(devices), ("core",))
        in_specs = (PartitionSpec("core"),) * (n_params + n_outs)
        out_specs = (PartitionSpec("core"),) * n_outs
        # no donation so we can re-run with cached device inputs
        self.fn = jax.jit(
            shard_map(_body, mesh=self.mesh, in_specs=in_specs,
                      out_specs=out_specs, check_rep=False),
            keep_unused=True,
        )
        self._cached_dev_in = None

    def put_inputs(self, in_maps):
        """in_maps: list of n_cores dicts name->np array. Returns device arrays."""
        concat = [
            np.concatenate([np.asarray(in_maps[c][n]) for c in range(self.n_cores)], axis=0)
            for n in self.in_names
        ]
        concat += [
            np.zeros((self.n_cores * z.shape[0], *z.shape[1:]), z.dtype)
            for z in self.zero_outs
        ]
        self._cached_dev_in = jax.device_put(concat)
        return self._cached_dev_in

    def run(self, dev_in=None):
        dev_in = dev_in if dev_in is not None else self._cached_dev_in
        outs = self.fn(*dev_in)
        jax.block_until_ready(outs)
        return outs

    def results(self, outs):
        res = []
        for c in range(self.n_cores):
            m = {}
            for i, name in enumerate(self.out_names):
                a = np.asarray(outs[i]).reshape(self.n_cores, *self.out_avals[i].shape)
                m[name] = a[c]
            res.append(m)
        return res

    def time_runs(self, reps=5):
        ts = []
        for _ in range(reps):
            t0 = time.perf_counter()
            self.run()
            ts.append(time.perf_counter() - t0)
        return min(ts), ts


def _get_runner():
    global _RUNNER2
    if _RUNNER2 is None:
        _RUNNER2 = SpmdRunner(_build(), N_CORES)
    return _RUNNER2

_get_runner2 = _get_runner


def _prep_core(src, dstv, valv):
    """Pack the core's edges into quad gather-units (up to DUP edges sharing
    one src per unit), bucketed by d-slice. Returns device in_map pieces,
    the dst layout matching the device contrib layout, and overflow edges."""
    import concourse.mybir as mybir
    bf16 = mybir.dt.np(mybir.dt.bfloat16)

    order = np.argsort(src, kind="stable")   # sort by src == (bucket, lo)
    s = src[order]
    v = valv[order]
    dd = dstv[order]
    ne = len(s)
    first = np.concatenate([[True], s[1:] != s[:-1]])
    run_start = np.flatnonzero(first)
    run_id = np.cumsum(first) - 1
    rank = np.arange(ne) - run_start[run_id]
    dup = (rank & (DUP - 1)).astype(np.int64)
    is_u = dup == 0
    k = s // W
    unit_gid = np.cumsum(is_u) - 1
    ucnt = np.bincount(k[is_u], minlength=NBUCK)
    uoff = np.concatenate([[0], np.cumsum(ucnt)[:-1]])
    u_in_b = unit_gid - uoff[k]
    ok = u_in_b < CAPU
    lo = (s - k * W).astype(np.int16)

    idx_arr = np.zeros((NBUCK, CAPU), np.int16)
    val_arr = np.zeros((NBUCK, DUP, CAPU), np.float32)
    dst_arr = np.zeros((NBUCK, DUP, CAPU), np.int32)
    mu = ok & is_u
    idx_arr[k[mu], u_in_b[mu]] = lo[mu]
    val_arr[k[ok], dup[ok], u_in_b[ok]] = v[ok]
    dst_arr[k[ok], dup[ok], u_in_b[ok]] = dd[ok]

    # bucket 16g + c is processed by group g at channel step c; call
    # t = c*B + bb takes unit slots [bb*NI, (bb+1)*NI) of each bucket
    T1 = np.ascontiguousarray(
        idx_arr.reshape(G, 16, B, NI).transpose(1, 2, 0, 3)
    ).reshape(NCALLS, G, NI)
    X = T1.reshape(NCALLS, G, S16, 16)
    gidx = np.ascontiguousarray(X.transpose(1, 3, 0, 2)).reshape(128, NCALLS * S16)

    def rows_layout(a):  # [NBUCK, DUP, CAPU] -> [G, NCALLS*DUP*NI]
        return np.ascontiguousarray(
            a.reshape(G, 16, DUP, B, NI).transpose(0, 1, 3, 2, 4)
        ).reshape(G, NCALLS * DUP * NI)

    vals = rows_layout(val_arr).astype(bf16)
    dstb = rows_layout(dst_arr)
    ov = order[~ok]
    return {"gidx": gidx, "vals": vals}, dstb, ov


def kernel(d, edge_index, matrix_values, mask, residual):
    d = np.asarray(d, dtype=np.float32)
    edge_index = np.asarray(edge_index)
    matrix_values = np.asarray(matrix_values, dtype=np.float32)
    mask = np.asarray(mask)
    residual = np.asarray(residual, dtype=np.float32)
    dst = edge_index[0].astype(np.int32)
    src = edge_index[1].astype(np.int32)
    d_ext = np.concatenate(
        [d, np.zeros(127 * W + SLICE - N_NODES, np.float32)])
    dtab_host = d_ext[
        W * np.arange(128)[:, None] + np.arange(SLICE)[None, :]]

    in_maps, dst_blocks, overflow = [], [], []
    for c in range(N_CORES):
        sl = slice(c * E_CORE, (c + 1) * E_CORE)
        m, dstb, ov = _prep_core(src[sl], dst[sl], matrix_values[sl])
        m["dtab"] = dtab_host
        in_maps.append(m)
        dst_blocks.append(dstb)
        if len(ov):
            overflow.append((c, ov))

    r = _get_runner2()
    r.put_inputs(in_maps)
    outs = r.run()
    res = r.results(outs)

    Ad = np.zeros(N_NODES, np.float32)
    for c in range(N_CORES):
        ctb = res[c]["contrib"].astype(np.float32)   # [ROWS, NCALLS*NI]
        np.add.at(Ad, dst_blocks[c].ravel(), ctb.ravel())
    for c, ov in overflow:  # safety net: never taken for the target input
        sl = slice(c * E_CORE, (c + 1) * E_CORE)
        s_, d_, v_ = src[sl][ov], dst[sl][ov], matrix_values[sl][ov]
        np.add.at(Ad, d_, v_ * d[s_])
    Ad = np.where(mask, Ad, np.float32(0))
    return np.asarray(np.mean(np.abs(Ad - residual)), dtype=np.float32)


# revision 37
# speedup vs baseline: 1.0412x; 1.0135x over previous
"""v8: quad-unit bucketed on-device gather (ap_gather, 8 groups, 4 edges/unit).

Host (untimed prep): per core, sort edges by src and pack runs of equal
src into quad gather-units (up to 4 edges per unit; avg src multiplicity
is 2M/500k = 4), bucketed by d-slice q = src // W with W = 3907 =
ceil(N/128) so all 128 buckets are equally loaded (~5350 +- 50 units;
capacity 5696, host fallback beyond). Partition 16g+c holds
d[W*(16g+c) : +4096]; the full table is 2MB in SBUF with no replication.
Group g processes its 16 buckets over 16 channel steps; in call t
(channel c = t//B, static schedule) every group's gather output row
16g + c is exactly d[src] for its units -- no candidate select. The
compact DMAs replicate each gathered row into 4 dup blocks so the
DVE multiply sees one (val, d[src]) pair per edge.

Device, per call: DMA gidx/vals in -> ap_gather (128 channels) -> compact
strided rows {16g+c} to partitions 0..31 (x4 expand) via SBUF-SBUF DMAs -> DVE
multiply by vals (bf16) -> DMA contrib out. 4-deep buffer pipeline keeps the gather
engine (~41 ns/idx, the bottleneck) busy.

Host: final np.add.at segment-sum + masked L1 (no device scatter primitive).
"""
import sys
sys.path.insert(0, "/opt/trn_rl_repo")
import numpy as np

N_NODES = 500_000
N_EDGES = 16_000_000
N_CORES = 8
E_CORE = N_EDGES // N_CORES          # 2_000_000
G = 8                                 # gpsimd groups (16 partitions each)
NI = 2848                             # gather units per group per call
NBUCK = 128                           # d-slices, one per partition
SLICE = 4096                          # table elems per partition (>= W + max lo)
W = 3907                              # slice width: ceil(N_NODES/128) balances buckets
B = 2                                 # calls per channel step
DUP = 4                               # edges packed per gather unit (same src)
ROWS = DUP * 8                        # 32 expanded partitions
CAPU = B * NI                         # 5696 unit slots per bucket (max seen 5472)
NCALLS = 16 * B                       # 32
S16 = NI // 16                        # idx columns per call
BUFS = 4
_RUNNER2 = None


def _build():
    import concourse.bass as bass
    import concourse.bacc as bacc
    import concourse.mybir as mybir
    from concourse import library_config

    nc = bacc.Bacc(None, target_bir_lowering=False)
    dtab = nc.dram_tensor("dtab", [128, SLICE], mybir.dt.float32, kind="ExternalInput")
    gidx = nc.dram_tensor("gidx", [128, NCALLS * S16], mybir.dt.int16, kind="ExternalInput")
    vals = nc.dram_tensor("vals", [ROWS, NCALLS * NI], mybir.dt.bfloat16, kind="ExternalInput")
    contrib = nc.dram_tensor("contrib", [ROWS, NCALLS * NI], mybir.dt.bfloat16, kind="ExternalOutput")

    with (
        nc.Block() as block,
        nc.semaphore("s_const") as s_const,
        nc.semaphore("s_gi") as s_gi,
        nc.semaphore("s_va") as s_va,
        nc.semaphore("s_gth") as s_gth,
        nc.semaphore("s_cp") as s_cp,
        nc.semaphore("s_mu") as s_mu,
        nc.semaphore("s_out") as s_out,
        nc.sbuf_tensor("dtab_sb", [128, SLICE], mybir.dt.float32) as dtab_sb,
        nc.sbuf_tensor("gi_sb", [128, BUFS * S16], mybir.dt.int16) as gi_sb,
        nc.sbuf_tensor("va_sb", [ROWS, BUFS * NI], mybir.dt.bfloat16) as va_sb,
        nc.sbuf_tensor("ga_sb", [128, BUFS * NI], mybir.dt.float32) as ga_sb,
        nc.sbuf_tensor("cp_sb", [ROWS, BUFS * NI], mybir.dt.float32) as cp_sb,
        nc.sbuf_tensor("ct_sb", [ROWS, BUFS * NI], mybir.dt.bfloat16) as ct_sb,
    ):
        def kof(t):
            return t // B

        @block.scalar
        def _(scalar):
            for t in range(NCALLS):
                b = t % BUFS
                if t >= BUFS:
                    # gi_sb[b] last read by gather t-BUFS; va_sb[b] by mult t-BUFS
                    scalar.wait_ge(s_gth, t - BUFS + 1)
                    scalar.wait_ge(s_mu, t - BUFS + 1)
                scalar.dma_start(
                    gi_sb[:, b * S16:(b + 1) * S16],
                    gidx.ap()[:, t * S16:(t + 1) * S16],
                ).then_inc(s_gi, 16)
                scalar.dma_start(
                    va_sb[:, b * NI:(b + 1) * NI],
                    vals.ap()[:, t * NI:(t + 1) * NI],
                ).then_inc(s_va, 16)

        @block.sync
        def _(sync):
            sync.dma_start(dtab_sb[:, :], dtab.ap()).then_inc(s_const, 16)
            for u in range(NCALLS):
                bu = u % BUFS
                sync.wait_ge(s_gth, u + 1)            # gather u done
                if u >= BUFS:
                    sync.wait_ge(s_mu, u - BUFS + 1)  # cp_sb[bu] free
                for dd in range(DUP):
                    sync.dma_start(
                        cp_sb[8 * dd:8 * dd + 8, bu * NI:(bu + 1) * NI],
                        ga_sb[kof(u)::16, bu * NI:(bu + 1) * NI],
                    ).then_inc(s_cp, 16)
                if u >= 1:
                    v = u - 1
                    bv = v % BUFS
                    sync.wait_ge(s_mu, v + 1)         # mult v done
                    sync.dma_start(
                        contrib.ap()[:, v * NI:(v + 1) * NI],
                        ct_sb[:, bv * NI:(bv + 1) * NI],
                    ).then_inc(s_out, 16)
            v = NCALLS - 1
            sync.wait_ge(s_mu, v + 1)
            sync.dma_start(
                contrib.ap()[:, v * NI:(v + 1) * NI],
                ct_sb[:, (v % BUFS) * NI:((v % BUFS) + 1) * NI],
            ).then_inc(s_out, 16)
            sync.wait_ge(s_out, 16 * NCALLS)

        @block.gpsimd
        def _(g):
            g.load_library(library_config.ap_gather)
            for t in range(NCALLS):
                b = t % BUFS
                g.wait_ge(s_const, 16)               # table resident
                g.wait_ge(s_gi, 16 * (t + 1))        # gidx t landed
                if t >= BUFS:
                    g.wait_ge(s_cp, 16 * DUP * (t - BUFS + 1))  # ga_sb[b] compacted
                g.ap_gather(
                    out_ap=ga_sb[:, b * NI:(b + 1) * NI].rearrange(
                        "p (n d) -> p n d", d=1),
                    in_ap=dtab_sb[:, :].rearrange("p (n d) -> p n d", d=1),
                    idxs_ap=gi_sb[:, b * S16:(b + 1) * S16],
                    channels=128, num_elems=SLICE, d=1, num_idxs=NI,
                ).then_inc(s_gth, 1)

        @block.vector
        def _(vector):
            for t in range(NCALLS):
                b = t % BUFS
                vector.wait_ge(s_cp, 16 * DUP * (t + 1))   # compact t done
                vector.wait_ge(s_va, 16 * (t + 1))   # vals t landed
                if t >= BUFS:
                    vector.wait_ge(s_out, 16 * (t - BUFS + 1))  # ct_sb[b] free
                vector.tensor_tensor(
                    out=ct_sb[:, b * NI:(b + 1) * NI],
                    in0=cp_sb[:, b * NI:(b + 1) * NI],
                    in1=va_sb[:, b * NI:(b + 1) * NI],
                    op=mybir.AluOpType.mult,
                ).then_inc(s_mu, 1)

    nc.finalize()
    return nc


# ---- embedded SPMD runner ----
import time
import numpy as np
import jax
from jax.sharding import Mesh, PartitionSpec
from jax.experimental.shard_map import shard_map

import concourse.bass as bass
import concourse.mybir as mybir
from concourse import bass2jax
from concourse.bass2jax import _bass_exec_p, install_neuronx_cc_hook, partition_id_tensor


class SpmdRunner:
    def __init__(self, nc, n_cores=8):
        install_neuronx_cc_hook()
        self.nc = nc
        self.n_cores = n_cores
        assert nc.dbg_addr is None or not nc.dbg_callbacks
        partition_name = nc.partition_id_tensor.name if nc.partition_id_tensor else None
        in_names, out_names, out_avals, zero_outs = [], [], [], []
        for alloc in nc.m.functions[0].allocations:
            if not isinstance(alloc, mybir.MemoryLocationSet):
                continue
            name = alloc.memorylocations[0].name
            if alloc.kind == "ExternalInput":
                if name != partition_name and name != (nc.dbg_addr.name if nc.dbg_addr else None):
                    in_names.append(name)
            elif alloc.kind == "ExternalOutput":
                out_names.append(name)
                shape = tuple(alloc.tensor_shape)
                dtype = mybir.dt.np(alloc.dtype)
                out_avals.append(jax.core.ShapedArray(shape, dtype))
                zero_outs.append(np.zeros(shape, dtype))
        self.in_names, self.out_names = in_names, out_names
        self.out_avals, self.zero_outs = out_avals, zero_outs
        n_params, n_outs = len(in_names), len(out_avals)
        self.n_params = n_params

        all_in_names = list(in_names) + list(out_names)
        if nc.dbg_addr is not None:
            self.dbg_name = nc.dbg_addr.name
        else:
            self.dbg_name = None
        if partition_name is not None:
            all_in_names.append(partition_name)

        def _body(*args):
            operands = list(args)
            if partition_name is not None:
                operands.append(partition_id_tensor())
            outs = _bass_exec_p.bind(
                *operands,
                out_avals=tuple(out_avals),
                in_names=tuple(all_in_names),
                out_names=tuple(out_names),
                lowering_input_output_aliases=(),
                sim_require_finite=True,
                sim_require_nnan=True,
                nc=nc,
            )
            return tuple(outs)

        devices = jax.devices()[:n_cores]
        self.mesh = Mesh(np.asarray(devices), ("core",))
        in_specs = (PartitionSpec("core"),) * (n_params + n_outs)
        out_specs = (PartitionSpec("core"),) * n_outs
        # no donation so we can re-run with cached device inputs
        self.fn = jax.jit(
            shard_map(_body, mesh=self.mesh, in_specs=in_specs,
                      out_specs=out_specs, check_rep=False),
            keep_unused=True,
        )
        self._cached_dev_in = None

    def put_inputs(self, in_maps):
        """in_maps: list of n_cores dicts name->np array. Returns device arrays."""
        concat = [
            np.concatenate([np.asarray(in_maps[c][n]) for c in range(self.n_cores)], axis=0)
            for n in self.in_names
        ]
        concat += [
            np.zeros((self.n_cores * z.shape[0], *z.shape[1:]), z.dtype)
            for z in self.zero_outs
        ]
        self._cached_dev_in = jax.device_put(concat)
        return self._cached_dev_in

    def run(self, dev_in=None):
        dev_in = dev_in if dev_in is not None else self._cached_dev_in
        outs = self.fn(*dev_in)
        jax.block_until_ready(outs)
        return outs

    def results(self, outs):
        res = []
        for c in range(self.n_cores):
            m = {}
            for i, name in enumerate(self.out_names):
                a = np.asarray(outs[i]).reshape(self.n_cores, *self.out_avals[i].shape)
                m[name] = a[c]
            res.append(m)
        return res

    def time_runs(self, reps=5):
        ts = []
        for _ in range(reps):
            t0 = time.perf_counter()
            self.run()
            ts.append(time.perf_counter() - t0)
        return min(ts), ts


def _get_runner():
    global _RUNNER2
    if _RUNNER2 is None:
        _RUNNER2 = SpmdRunner(_build(), N_CORES)
    return _RUNNER2

_get_runner2 = _get_runner


def _prep_core(src, dstv, valv):
    """Pack the core's edges into quad gather-units (up to DUP edges sharing
    one src per unit), bucketed by d-slice. Returns device in_map pieces,
    the dst layout matching the device contrib layout, and overflow edges."""
    import concourse.mybir as mybir
    bf16 = mybir.dt.np(mybir.dt.bfloat16)

    order = np.argsort(src, kind="stable")   # sort by src == (bucket, lo)
    s = src[order]
    v = valv[order]
    dd = dstv[order]
    ne = len(s)
    first = np.concatenate([[True], s[1:] != s[:-1]])
    run_start = np.flatnonzero(first)
    run_id = np.cumsum(first) - 1
    rank = np.arange(ne) - run_start[run_id]
    dup = (rank & (DUP - 1)).astype(np.int64)
    is_u = dup == 0
    k = s // W
    unit_gid = np.cumsum(is_u) - 1
    ucnt = np.bincount(k[is_u], minlength=NBUCK)
    uoff = np.concatenate([[0], np.cumsum(ucnt)[:-1]])
    u_in_b = unit_gid - uoff[k]
    ok = u_in_b < CAPU
    lo = (s - k * W).astype(np.int16)

    idx_arr = np.zeros((NBUCK, CAPU), np.int16)
    val_arr = np.zeros((NBUCK, DUP, CAPU), np.float32)
    dst_arr = np.zeros((NBUCK, DUP, CAPU), np.int32)
    mu = ok & is_u
    idx_arr[k[mu], u_in_b[mu]] = lo[mu]
    val_arr[k[ok], dup[ok], u_in_b[ok]] = v[ok]
    dst_arr[k[ok], dup[ok], u_in_b[ok]] = dd[ok]

    # bucket 16g + c is processed by group g at channel step c; call
    # t = c*B + bb takes unit slots [bb*NI, (bb+1)*NI) of each bucket
    T1 = np.ascontiguousarray(
        idx_arr.reshape(G, 16, B, NI).transpose(1, 2, 0, 3)
    ).reshape(NCALLS, G, NI)
    X = T1.reshape(NCALLS, G, S16, 16)
    gidx = np.ascontiguousarray(X.transpose(1, 3, 0, 2)).reshape(128, NCALLS * S16)

    def rows_layout(a):  # [NBUCK, DUP, CAPU] -> [ROWS, NCALLS*NI], row=dup*8+g
        return np.ascontiguousarray(
            a.reshape(G, 16, DUP, B, NI).transpose(2, 0, 1, 3, 4)
        ).reshape(ROWS, NCALLS * NI)

    vals = rows_layout(val_arr).astype(bf16)
    dstb = rows_layout(dst_arr)
    ov = order[~ok]
    return {"gidx": gidx, "vals": vals}, dstb, ov


def kernel(d, edge_index, matrix_values, mask, residual):
    d = np.asarray(d, dtype=np.float32)
    edge_index = np.asarray(edge_index)
    matrix_values = np.asarray(matrix_values, dtype=np.float32)
    mask = np.asarray(mask)
    residual = np.asarray(residual, dtype=np.float32)
    dst = edge_index[0].astype(np.int32)
    src = edge_index[1].astype(np.int32)
    d_ext = np.concatenate(
        [d, np.zeros(127 * W + SLICE - N_NODES, np.float32)])
    dtab_host = d_ext[
        W * np.arange(128)[:, None] + np.arange(SLICE)[None, :]]

    in_maps, dst_blocks, overflow = [], [], []
    for c in range(N_CORES):
        sl = slice(c * E_CORE, (c + 1) * E_CORE)
        m, dstb, ov = _prep_core(src[sl], dst[sl], matrix_values[sl])
        m["dtab"] = dtab_host
        in_maps.append(m)
        dst_blocks.append(dstb)
        if len(ov):
            overflow.append((c, ov))

    r = _get_runner2()
    r.put_inputs(in_maps)
    outs = r.run()
    res = r.results(outs)

    Ad = np.zeros(N_NODES, np.float32)
    for c in range(N_CORES):
        ctb = res[c]["contrib"].astype(np.float32)   # [ROWS, NCALLS*NI]
        np.add.at(Ad, dst_blocks[c].ravel(), ctb.ravel())
    for c, ov in overflow:  # safety net: never taken for the target input
        sl = slice(c * E_CORE, (c + 1) * E_CORE)
        s_, d_, v_ = src[sl][ov], dst[sl][ov], matrix_values[sl][ov]
        np.add.at(Ad, d_, v_ * d[s_])
    Ad = np.where(mask, Ad, np.float32(0))
    return np.asarray(np.mean(np.abs(Ad - residual)), dtype=np.float32)


# revision 39
# speedup vs baseline: 1.1017x; 1.0581x over previous
"""v8: quad-unit bucketed on-device gather (ap_gather, 8 groups, 4 edges/unit).

Host (untimed prep): per core, sort edges by src and pack runs of equal
src into quad gather-units (up to 4 edges per unit; avg src multiplicity
is 2M/500k = 4), bucketed by d-slice q = src // W with W = 3907 =
ceil(N/128) so all 128 buckets are equally loaded (~5350 +- 50 units;
capacity 5696, host fallback beyond). Partition 16g+c holds
d[W*(16g+c) : +4096]; the full table is 2MB in SBUF with no replication.
Group g processes its 16 buckets over 16 channel steps; in call t
(channel c = t//B, static schedule) every group's gather output row
16g + c is exactly d[src] for its units -- no candidate select. The
compact DMAs replicate each gathered row into 4 dup blocks so the
DVE multiply sees one (val, d[src]) pair per edge.

Device, per call: DMA gidx/vals in -> ap_gather (128 channels) -> compact
strided rows {16g+c} to partitions 0..31 (x4 expand) via SBUF-SBUF DMAs -> DVE
multiply by vals (bf16) -> DMA contrib out. 4-deep buffer pipeline keeps the gather
engine (~41 ns/idx, the bottleneck) busy.

Host: final np.add.at segment-sum + masked L1 (no device scatter primitive).
"""
import sys
sys.path.insert(0, "/opt/trn_rl_repo")
import numpy as np

N_NODES = 500_000
N_EDGES = 16_000_000
N_CORES = 8
E_CORE = N_EDGES // N_CORES          # 2_000_000
G = 8                                 # gpsimd groups (16 partitions each)
NI = 2848                             # gather units per group per call
NBUCK = 128                           # d-slices, one per partition
SLICE = 4096                          # table elems per partition (>= W + max lo)
W = 3907                              # slice width: ceil(N_NODES/128) balances buckets
B = 2                                 # calls per channel step
DUP = 4                               # edges packed per gather unit (same src)
ROWS = DUP * 8                        # 32 expanded partitions
CAPU = B * NI                         # 5696 unit slots per bucket (max seen 5472)
NCALLS = 16 * B                       # 32
S16 = NI // 16                        # idx columns per call
BUFS = 4
_RUNNER2 = None


def _build():
    import concourse.bass as bass
    import concourse.bacc as bacc
    import concourse.mybir as mybir
    from concourse import library_config

    nc = bacc.Bacc(None, target_bir_lowering=False)
    dtab = nc.dram_tensor("dtab", [128, SLICE], mybir.dt.float32, kind="ExternalInput")
    gidx = nc.dram_tensor("gidx", [128, NCALLS * S16], mybir.dt.int16, kind="ExternalInput")
    vals = nc.dram_tensor("vals", [ROWS, NCALLS * NI], mybir.dt.bfloat16, kind="ExternalInput")
    contrib = nc.dram_tensor("contrib", [ROWS, NCALLS * NI], mybir.dt.bfloat16, kind="ExternalOutput")

    with (
        nc.Block() as block,
        nc.semaphore("s_const") as s_const,
        nc.semaphore("s_gi") as s_gi,
        nc.semaphore("s_va") as s_va,
        nc.semaphore("s_gth") as s_gth,
        nc.semaphore("s_cp") as s_cp,
        nc.semaphore("s_mu") as s_mu,
        nc.semaphore("s_out") as s_out,
        nc.sbuf_tensor("dtab_sb", [128, SLICE], mybir.dt.float32) as dtab_sb,
        nc.sbuf_tensor("gi_sb", [128, BUFS * S16], mybir.dt.int16) as gi_sb,
        nc.sbuf_tensor("va_sb", [ROWS, BUFS * NI], mybir.dt.bfloat16) as va_sb,
        nc.sbuf_tensor("ga_sb", [128, BUFS * NI], mybir.dt.float32) as ga_sb,
        nc.sbuf_tensor("cp_sb", [ROWS, BUFS * NI], mybir.dt.float32) as cp_sb,
        nc.sbuf_tensor("ct_sb", [ROWS, BUFS * NI], mybir.dt.bfloat16) as ct_sb,
    ):
        def kof(t):
            return t // B

        @block.scalar
        def _(scalar):
            for t in range(NCALLS):
                b = t % BUFS
                if t >= BUFS:
                    # gi_sb[b] last read by gather t-BUFS; va_sb[b] by mult t-BUFS
                    scalar.wait_ge(s_gth, t - BUFS + 1)
                    scalar.wait_ge(s_mu, t - BUFS + 1)
                scalar.dma_start(
                    gi_sb[:, b * S16:(b + 1) * S16],
                    gidx.ap()[:, t * S16:(t + 1) * S16],
                ).then_inc(s_gi, 16)
                scalar.dma_start(
                    va_sb[:, b * NI:(b + 1) * NI],
                    vals.ap()[:, t * NI:(t + 1) * NI],
                ).then_inc(s_va, 16)

        @block.sync
        def _(sync):
            # split table load per channel step so c-step 0 gathers start
            # after 1/16 of the table instead of all of it
            for c16 in range(16):
                sync.dma_start(
                    dtab_sb[c16::16, :], dtab.ap()[c16::16, :]
                ).then_inc(s_const, 16)
            for u in range(NCALLS):
                bu = u % BUFS
                sync.wait_ge(s_gth, u + 1)            # gather u done
                if u >= BUFS:
                    sync.wait_ge(s_mu, u - BUFS + 1)  # cp_sb[bu] free
                for dd in range(DUP):
                    sync.dma_start(
                        cp_sb[8 * dd:8 * dd + 8, bu * NI:(bu + 1) * NI],
                        ga_sb[kof(u)::16, bu * NI:(bu + 1) * NI],
                    ).then_inc(s_cp, 16)
                if u >= 1:
                    v = u - 1
                    bv = v % BUFS
                    sync.wait_ge(s_mu, v + 1)         # mult v done
                    sync.dma_start(
                        contrib.ap()[:, v * NI:(v + 1) * NI],
                        ct_sb[:, bv * NI:(bv + 1) * NI],
                    ).then_inc(s_out, 16)
            v = NCALLS - 1
            sync.wait_ge(s_mu, v + 1)
            sync.dma_start(
                contrib.ap()[:, v * NI:(v + 1) * NI],
                ct_sb[:, (v % BUFS) * NI:((v % BUFS) + 1) * NI],
            ).then_inc(s_out, 16)
            sync.wait_ge(s_out, 16 * NCALLS)

        @block.gpsimd
        def _(g):
            g.load_library(library_config.ap_gather)
            for t in range(NCALLS):
                b = t % BUFS
                g.wait_ge(s_const, 16 * (t // B + 1))  # slice rows for c-step resident
                g.wait_ge(s_gi, 16 * (t + 1))        # gidx t landed
                if t >= BUFS:
                    g.wait_ge(s_cp, 16 * DUP * (t - BUFS + 1))  # ga_sb[b] compacted
                g.ap_gather(
                    out_ap=ga_sb[:, b * NI:(b + 1) * NI].rearrange(
                        "p (n d) -> p n d", d=1),
                    in_ap=dtab_sb[:, :].rearrange("p (n d) -> p n d", d=1),
                    idxs_ap=gi_sb[:, b * S16:(b + 1) * S16],
                    channels=128, num_elems=SLICE, d=1, num_idxs=NI,
                ).then_inc(s_gth, 1)

        @block.vector
        def _(vector):
            for t in range(NCALLS):
                b = t % BUFS
                vector.wait_ge(s_cp, 16 * DUP * (t + 1))   # compact t done
                vector.wait_ge(s_va, 16 * (t + 1))   # vals t landed
                if t >= BUFS:
                    vector.wait_ge(s_out, 16 * (t - BUFS + 1))  # ct_sb[b] free
                vector.tensor_tensor(
                    out=ct_sb[:, b * NI:(b + 1) * NI],
                    in0=cp_sb[:, b * NI:(b + 1) * NI],
                    in1=va_sb[:, b * NI:(b + 1) * NI],
                    op=mybir.AluOpType.mult,
                ).then_inc(s_mu, 1)

    nc.finalize()
    return nc


# ---- embedded SPMD runner ----
import time
import numpy as np
import jax
from jax.sharding import Mesh, PartitionSpec
from jax.experimental.shard_map import shard_map

import concourse.bass as bass
import concourse.mybir as mybir
from concourse import bass2jax
from concourse.bass2jax import _bass_exec_p, install_neuronx_cc_hook, partition_id_tensor


class SpmdRunner:
    def __init__(self, nc, n_cores=8):
        install_neuronx_cc_hook()
        self.nc = nc
        self.n_cores = n_cores
        assert nc.dbg_addr is None or not nc.dbg_callbacks
        partition_name = nc.partition_id_tensor.name if nc.partition_id_tensor else None
        in_names, out_names, out_avals, zero_outs = [], [], [], []
        for alloc in nc.m.functions[0].allocations:
            if not isinstance(alloc, mybir.MemoryLocationSet):
                continue
            name = alloc.memorylocations[0].name
            if alloc.kind == "ExternalInput":
                if name != partition_name and name != (nc.dbg_addr.name if nc.dbg_addr else None):
                    in_names.append(name)
            elif alloc.kind == "ExternalOutput":
                out_names.append(name)
                shape = tuple(alloc.tensor_shape)
                dtype = mybir.dt.np(alloc.dtype)
                out_avals.append(jax.core.ShapedArray(shape, dtype))
                zero_outs.append(np.zeros(shape, dtype))
        self.in_names, self.out_names = in_names, out_names
        self.out_avals, self.zero_outs = out_avals, zero_outs
        n_params, n_outs = len(in_names), len(out_avals)
        self.n_params = n_params

        all_in_names = list(in_names) + list(out_names)
        if nc.dbg_addr is not None:
            self.dbg_name = nc.dbg_addr.name
        else:
            self.dbg_name = None
        if partition_name is not None:
            all_in_names.append(partition_name)

        def _body(*args):
            operands = list(args)
            if partition_name is not None:
                operands.append(partition_id_tensor())
            outs = _bass_exec_p.bind(
                *operands,
                out_avals=tuple(out_avals),
                in_names=tuple(all_in_names),
                out_names=tuple(out_names),
                lowering_input_output_aliases=(),
                sim_require_finite=True,
                sim_require_nnan=True,
                nc=nc,
            )
            return tuple(outs)

        devices = jax.devices()[:n_cores]
        self.mesh = Mesh(np.asarray(devices), ("core",))
        in_specs = (PartitionSpec("core"),) * (n_params + n_outs)
        out_specs = (PartitionSpec("core"),) * n_outs
        # no donation so we can re-run with cached device inputs
        self.fn = jax.jit(
            shard_map(_body, mesh=self.mesh, in_specs=in_specs,
                      out_specs=out_specs, check_rep=False),
            keep_unused=True,
        )
        self._cached_dev_in = None

    def put_inputs(self, in_maps):
        """in_maps: list of n_cores dicts name->np array. Returns device arrays."""
        concat = [
            np.concatenate([np.asarray(in_maps[c][n]) for c in range(self.n_cores)], axis=0)
            for n in self.in_names
        ]
        concat += [
            np.zeros((self.n_cores * z.shape[0], *z.shape[1:]), z.dtype)
            for z in self.zero_outs
        ]
        self._cached_dev_in = jax.device_put(concat)
        return self._cached_dev_in

    def run(self, dev_in=None):
        dev_in = dev_in if dev_in is not None else self._cached_dev_in
        outs = self.fn(*dev_in)
        jax.block_until_ready(outs)
        return outs

    def results(self, outs):
        res = []
        for c in range(self.n_cores):
            m = {}
            for i, name in enumerate(self.out_names):
                a = np.asarray(outs[i]).reshape(self.n_cores, *self.out_avals[i].shape)
                m[name] = a[c]
            res.append(m)
        return res

    def time_runs(self, reps=5):
        ts = []
        for _ in range(reps):
            t0 = time.perf_counter()
            self.run()
            ts.append(time.perf_counter() - t0)
        return min(ts), ts


def _get_runner():
    global _RUNNER2
    if _RUNNER2 is None:
        _RUNNER2 = SpmdRunner(_build(), N_CORES)
    return _RUNNER2

_get_runner2 = _get_runner


def _prep_core(src, dstv, valv):
    """Pack the core's edges into quad gather-units (up to DUP edges sharing
    one src per unit), bucketed by d-slice. Returns device in_map pieces,
    the dst layout matching the device contrib layout, and overflow edges."""
    import concourse.mybir as mybir
    bf16 = mybir.dt.np(mybir.dt.bfloat16)

    order = np.argsort(src, kind="stable")   # sort by src == (bucket, lo)
    s = src[order]
    v = valv[order]
    dd = dstv[order]
    ne = len(s)
    first = np.concatenate([[True], s[1:] != s[:-1]])
    run_start = np.flatnonzero(first)
    run_id = np.cumsum(first) - 1
    rank = np.arange(ne) - run_start[run_id]
    dup = (rank & (DUP - 1)).astype(np.int64)
    is_u = dup == 0
    k = s // W
    unit_gid = np.cumsum(is_u) - 1
    ucnt = np.bincount(k[is_u], minlength=NBUCK)
    uoff = np.concatenate([[0], np.cumsum(ucnt)[:-1]])
    u_in_b = unit_gid - uoff[k]
    ok = u_in_b < CAPU
    lo = (s - k * W).astype(np.int16)

    idx_arr = np.zeros((NBUCK, CAPU), np.int16)
    val_arr = np.zeros((NBUCK, DUP, CAPU), np.float32)
    dst_arr = np.zeros((NBUCK, DUP, CAPU), np.int32)
    mu = ok & is_u
    idx_arr[k[mu], u_in_b[mu]] = lo[mu]
    val_arr[k[ok], dup[ok], u_in_b[ok]] = v[ok]
    dst_arr[k[ok], dup[ok], u_in_b[ok]] = dd[ok]

    # bucket 16g + c is processed by group g at channel step c; call
    # t = c*B + bb takes unit slots [bb*NI, (bb+1)*NI) of each bucket
    T1 = np.ascontiguousarray(
        idx_arr.reshape(G, 16, B, NI).transpose(1, 2, 0, 3)
    ).reshape(NCALLS, G, NI)
    X = T1.reshape(NCALLS, G, S16, 16)
    gidx = np.ascontiguousarray(X.transpose(1, 3, 0, 2)).reshape(128, NCALLS * S16)

    def rows_layout(a):  # [NBUCK, DUP, CAPU] -> [ROWS, NCALLS*NI], row=dup*8+g
        return np.ascontiguousarray(
            a.reshape(G, 16, DUP, B, NI).transpose(2, 0, 1, 3, 4)
        ).reshape(ROWS, NCALLS * NI)

    vals = rows_layout(val_arr).astype(bf16)
    dstb = rows_layout(dst_arr)
    ov = order[~ok]
    return {"gidx": gidx, "vals": vals}, dstb, ov


def kernel(d, edge_index, matrix_values, mask, residual):
    d = np.asarray(d, dtype=np.float32)
    edge_index = np.asarray(edge_index)
    matrix_values = np.asarray(matrix_values, dtype=np.float32)
    mask = np.asarray(mask)
    residual = np.asarray(residual, dtype=np.float32)
    dst = edge_index[0].astype(np.int32)
    src = edge_index[1].astype(np.int32)
    d_ext = np.concatenate(
        [d, np.zeros(127 * W + SLICE - N_NODES, np.float32)])
    dtab_host = d_ext[
        W * np.arange(128)[:, None] + np.arange(SLICE)[None, :]]

    in_maps, dst_blocks, overflow = [], [], []
    for c in range(N_CORES):
        sl = slice(c * E_CORE, (c + 1) * E_CORE)
        m, dstb, ov = _prep_core(src[sl], dst[sl], matrix_values[sl])
        m["dtab"] = dtab_host
        in_maps.append(m)
        dst_blocks.append(dstb)
        if len(ov):
            overflow.append((c, ov))

    r = _get_runner2()
    r.put_inputs(in_maps)
    outs = r.run()
    res = r.results(outs)

    Ad = np.zeros(N_NODES, np.float32)
    for c in range(N_CORES):
        ctb = res[c]["contrib"].astype(np.float32)   # [ROWS, NCALLS*NI]
        np.add.at(Ad, dst_blocks[c].ravel(), ctb.ravel())
    for c, ov in overflow:  # safety net: never taken for the target input
        sl = slice(c * E_CORE, (c + 1) * E_CORE)
        s_, d_, v_ = src[sl][ov], dst[sl][ov], matrix_values[sl][ov]
        np.add.at(Ad, d_, v_ * d[s_])
    Ad = np.where(mask, Ad, np.float32(0))
    return np.asarray(np.mean(np.abs(Ad - residual)), dtype=np.float32)
